# revision 17
# baseline (speedup 1.0000x reference)
"""Trainium2 Bass kernel for nn_EncoderPrecision.

Math: two tiny MLPs map x (B,N,Dx) -> (B,N,Dz); vectors d=exp(mlp_d),
u=mlp_o give structurally sparse outputs with closed-form bands:
  D[i,i] = d_i;  Bmat = D + superdiag(u[:-1]);
  precision[i,i] = d_i^2 + u_{i-1}^2 + eps;  precision[i,i+1] = d_i u_i.

Sharding: data-parallel over batch B=8, one batch element per core;
weights replicated. The device computes every band VALUE (d, u, poff,
pdiag) and ships them as one compact (128, NT, 32) image per core; the
host places those values into dense zero backgrounds (pure layout).

Layout: the HOST pre-transposes x to xT (Dx, N) bf16 so tokens live on
the free dim end-to-end (no device transposes). All matmul operands are
bf16 (end-to-end rel_l2 ~4e-3 vs the 2e-2 gate); PSUM accumulation and
band math stay f32. Inputs arrive via parallel DMA paths (SP-HWDGE:
x+W0 split in two, ACT-HWDGE: W1+W2, Pool-SWDGE: biases+eps). b2 and
the EPS constant are preloaded into PSUM by tiny PE matmuls (ones-row
stationary) so the L3/shift matmuls accumulate onto them and the
ACT/DVE drain engines (the throughput bottleneck alongside PE) carry no
extra copies. The superdiagonal shift u2[a-1] (token a = t*128+p,
tile-major) is a partition shift done with two tiny PE matmuls
(sub-diagonal mask + row127->col0 seam mask). PSUM banks are managed
explicitly (8 x [128,512] tiles). Output ships per 512-token chunk:
chunk 0 via Pool-SWDGE (keeps HWDGE free), chunk 1 via SP.
"""

import numpy as np

EPS = 0.001
B, N, Dx, H, Dz = 8, 1024, 32, 256, 8
NCORES = 8
P = 128
NT = N // P          # 8 token tiles of 128
CH = 512             # chunk (psum bank limit)
NC = N // CH         # 2 chunks
TC = CH // P         # 4 tiles per chunk
KH = H // P          # 2 contraction tiles for H=256

_WEIGHT_SHAPES = {
    "dW0": (Dx, H), "db0": (H,), "dW1": (H, H), "db1": (H,),
    "dW2": (H, Dz), "db2": (Dz,),
    "oW0": (Dx, H), "ob0": (H,), "oW1": (H, H), "ob1": (H,),
    "oW2": (H, Dz), "ob2": (Dz,),
}

# bx0 bf16 (32, 1024): xT[:, 0:512] | w0d(256) | w0o(256)
# bx1 bf16 (32, 512):  xT[:, 512:1024]
# bw  bf16 (128, 1056): w1d(512) | w1o(512) | w2d(16) | w2o(16)
# bb0 f32  (128, 4):   b0d(2) b0o(2)            [first: gates L1 relus]
# bb1 f32  (128, 100): b1d(2) b1o(2) b2bc(64) epsbc(32)
BW_C = 512 + 512 + 16 + 16
BB1_C = 2 + 2 + 64 + 32


def _bf16():
    import ml_dtypes
    return np.dtype(ml_dtypes.bfloat16)


def _pack_weights(w):
    bf16 = _bf16()
    bw = np.zeros((P, BW_C), bf16)
    bw[:, 0:512] = w["dW1"].reshape(KH, P, H).transpose(1, 0, 2).reshape(P, 512).astype(bf16)
    bw[:, 512:1024] = w["oW1"].reshape(KH, P, H).transpose(1, 0, 2).reshape(P, 512).astype(bf16)
    bw[:, 1024:1040] = w["dW2"].reshape(KH, P, Dz).transpose(1, 0, 2).reshape(P, 16).astype(bf16)
    bw[:, 1040:1056] = w["oW2"].reshape(KH, P, Dz).transpose(1, 0, 2).reshape(P, 16).astype(bf16)
    bb0 = np.zeros((P, 4), np.float32)
    bb0[:, 0:2] = w["db0"].reshape(KH, P).T
    bb0[:, 2:4] = w["ob0"].reshape(KH, P).T
    bb1 = np.zeros((P, BB1_C), np.float32)
    bb1[:, 0:2] = w["db1"].reshape(KH, P).T
    bb1[:, 2:4] = w["ob1"].reshape(KH, P).T
    b2bc = np.concatenate([w["db2"], w["ob2"]])  # (16,)
    bb1[:, 4:68] = np.tile(b2bc, 4)[None, :]
    bb1[:, 68:100] = EPS
    return bw, bb0, bb1


def _pack_x(xi, w):
    bf16 = _bf16()
    xt = np.ascontiguousarray(xi.T).astype(bf16)
    bx0 = np.zeros((Dx, 1024), bf16)
    bx0[:, 0:512] = xt[:, 0:512]
    bx0[:, 512:768] = w["dW0"].astype(bf16)
    bx0[:, 768:1024] = w["oW0"].astype(bf16)
    bx1 = np.ascontiguousarray(xt[:, 512:1024])
    return bx0, bx1


def _emit(ctx, tc, nc, aps):
    import concourse.mybir as mybir

    f32 = mybir.dt.float32
    bf = mybir.dt.bfloat16
    AF = mybir.ActivationFunctionType

    const = ctx.enter_context(tc.tile_pool(name="const", bufs=1))
    work = ctx.enter_context(tc.tile_pool(name="work", bufs=1))
    bands = ctx.enter_context(tc.tile_pool(name="bands", bufs=1))
    psum = ctx.enter_context(tc.tile_pool(name="psum", bufs=1, space="PSUM"))

    pb = [psum.tile([P, CH], f32, tag=f"pb{i}", name=f"pb{i}")
          for i in range(8)]

    # warmup: set pe_busy_start early so the 3us p-state ramp completes
    # before the real matmul stream begins
    zp = const.tile([1, 8], f32)
    nc.vector.memset(zp[:], 0.0)
    wps = pb[7]
    nc.tensor.matmul(wps[:1, 0:8], zp[:, 0:1], zp[:], start=True, stop=True)

    # --- input DMAs on parallel issue paths, x+w0 first ---
    bx_sb = const.tile([Dx, 1024], bf)   # xT[:, 0:512] | w0d | w0o
    x1_sb = const.tile([Dx, 512], bf)    # xT[:, 512:1024]
    nc.sync.dma_start(bx_sb[:], aps["bx0"][:])
    nc.scalar.dma_start(x1_sb[:], aps["bx1"][:])                 # ACT HWDGE
    bw_sb = const.tile([P, BW_C], bf)
    nc.sync.dma_start(bw_sb[:], aps["bw"][:])                    # SP second
    bb0_sb = const.tile([P, 4], f32)
    nc.gpsimd.dma_start(bb0_sb[:], aps["bb0"][:])                # Pool SWDGE
    bb_sb = const.tile([P, BB1_C], f32)
    nc.gpsimd.dma_start(bb_sb[:], aps["bb1"][:])

    xv = [bx_sb[:, 0:512], x1_sb[:]]
    w0 = {"d": bx_sb[:, 512:768], "o": bx_sb[:, 768:1024]}
    w1 = {"d": bw_sb[:, 0:512].rearrange("p (ko m) -> p ko m", ko=KH),
          "o": bw_sb[:, 512:1024].rearrange("p (ko m) -> p ko m", ko=KH)}
    w2 = {"d": bw_sb[:, 1024:1040].rearrange("p (ko m) -> p ko m", ko=KH),
          "o": bw_sb[:, 1040:1056].rearrange("p (ko m) -> p ko m", ko=KH)}
    b0 = {"d": bb0_sb[:, 0:2], "o": bb0_sb[:, 2:4]}
    b1 = {"d": bb_sb[:, 0:2], "o": bb_sb[:, 2:4]}

    # masks (Pool, after its DMA issue; needed only at band time)
    # SH[k, m] = 1 iff k = m-1  ->  (SH^T v)[m] = v[m-1], 0 at m=0
    sh = const.tile([P, P], f32)
    nc.gpsimd.memset(sh[:], 0.0)
    nc.gpsimd.affine_select(
        out=sh[:], in_=sh[:], compare_op=mybir.AluOpType.not_equal,
        fill=1.0, base=1, pattern=[[-1, P]], channel_multiplier=1)
    # S127[k, m] = 1 iff k = 127 and m = 0  (tile-seam: prev tile's p=127)
    s127 = const.tile([P, P], f32)
    nc.gpsimd.memset(s127[:], 0.0)
    nc.gpsimd.affine_select(
        out=s127[:], in_=s127[:], compare_op=mybir.AluOpType.not_equal,
        fill=1.0, base=127, pattern=[[1, P]], channel_multiplier=-1)
    ones = const.tile([1, P], f32)
    nc.gpsimd.memset(ones[:], 1.0)

    h0, h1 = {}, {}
    for br in ("d", "o"):
        h0[br] = work.tile([P, KH, N], bf, tag=f"h0{br}", name=f"h0{br}")
        h1[br] = work.tile([P, KH, N], bf, tag=f"h1{br}", name=f"h1{br}")

    obig = bands.tile([P, NT, 32], f32)   # d | u | pdiag | poff per token
    d2 = bands.tile([P, NT, Dz], f32)
    u2 = bands.tile([P, NT, Dz], f32)

    def relu_store(eng, dst, ps, bias):
        if eng == "A":
            nc.scalar.activation(dst, ps, AF.Relu, bias=bias)
        else:
            nc.vector.tensor_scalar(
                dst, ps, bias, 0.0, mybir.AluOpType.add, mybir.AluOpType.max)

    def emit_l1(c):
        ns = slice(c * CH, (c + 1) * CH)
        for bi, br in enumerate(("d", "o")):
            for m in range(KH):
                ps = pb[4 * c + 2 * bi + m]
                nc.tensor.matmul(
                    ps[:], w0[br][:, m * P:(m + 1) * P], xv[c][:],
                    start=True, stop=True)
                relu_store("A" if m == 0 else "D",
                           h0[br][:, m, ns], ps[:], b0[br][:, m:m + 1])

    def emit_l2(c):
        ns = slice(c * CH, (c + 1) * CH)
        for bi, br in enumerate(("d", "o")):
            for m in range(KH):
                ps = pb[4 * c + 2 * bi + m]
                for k in range(KH):
                    nc.tensor.matmul(
                        ps[:], w1[br][:, k, m * P:(m + 1) * P],
                        h0[br][:, k, ns],
                        start=(k == 0), stop=(k == KH - 1))
                relu_store("A" if m == 0 else "D",
                           h1[br][:, m, ns], ps[:], b1[br][:, m:m + 1])

    def emit_l3(c):
        # b2 preload by PE (ones-row matmul), then head matmuls accumulate
        yv = pb[2 * c][:, 0:TC * 16].rearrange("p (t z) -> p t z", z=16)
        nc.tensor.matmul(yv[:], ones[:], bb_sb[0:1, 4:68],
                         start=True, stop=False)
        for tt in range(TC):
            ta = c * TC + tt
            for bi, br in enumerate(("d", "o")):
                zs = slice(bi * Dz, (bi + 1) * Dz)
                for k in range(KH):
                    nc.tensor.matmul(
                        yv[:, tt, zs], h1[br][:, k, ta * P:(ta + 1) * P],
                        w2[br][:, k, :],
                        start=False, stop=(k == KH - 1))
        return yv

    def emit_shift(c):
        # sps = EPS + u2 shifted by one token (partition shift + tile seam)
        sv = pb[2 * c + 1][:, 0:TC * Dz].rearrange("p (t z) -> p t z", z=Dz)
        nc.tensor.matmul(sv[:], ones[:], bb_sb[0:1, 68:100],
                         start=True, stop=False)
        ts = slice(c * TC, (c + 1) * TC)
        nc.tensor.matmul(sv[:, 0:TC, :], sh[:], u2[:, ts, :],
                         start=False, stop=False)
        if c == 0:
            nc.tensor.matmul(sv[:, 1:TC, :], s127[:], u2[:, 0:TC - 1, :],
                             start=False, stop=True)
        else:
            nc.tensor.matmul(sv[:, 0:TC, :], s127[:],
                             u2[:, c * TC - 1:(c + 1) * TC - 1, :],
                             start=False, stop=True)
        return sv

    import bass_rust as _br

    prev = {}

    def pin(key, bi):
        # nosync edge: same-engine queue-order pin across band ops
        if key in prev:
            _br.add_dep_helper(bi.ins, prev[key].ins, sync=False,
                               reason="band order pin")
        prev[key] = bi

    def emit_bands(c, yv):
        # ACT: d, d2 (both Exp); DVE: u copy-out, pdiag; Pool: u2, poff.
        ts = slice(c * TC, (c + 1) * TC)
        pin("A", nc.scalar.activation(obig[:, ts, 0:Dz], yv[:, :, 0:Dz], AF.Exp))
        pin("V", nc.vector.tensor_copy(obig[:, ts, Dz:16], yv[:, :, Dz:16]))
        pin("P", nc.gpsimd.tensor_mul(
            u2[:, ts, :], obig[:, ts, Dz:16], obig[:, ts, Dz:16]))
        sv = emit_shift(c)
        pin("A", nc.scalar.activation(d2[:, ts, :], yv[:, :, 0:Dz], AF.Exp,
                                      scale=2.0))
        pin("P", nc.gpsimd.tensor_mul(
            obig[:, ts, 24:32], obig[:, ts, 0:Dz], obig[:, ts, Dz:16]))
        pin("V", nc.vector.tensor_tensor(
            obig[:, ts, 16:24], d2[:, ts, :], sv[:, 0:TC, :],
            mybir.AluOpType.add))

    emit_l1(0)
    emit_l1(1)
    emit_l2(0)
    emit_l2(1)
    yv0 = emit_l3(0)
    emit_bands(0, yv0)
    yv1 = emit_l3(1)
    emit_bands(1, yv1)
    nc.sync.dma_start(aps["out"][:], obig[:])


def _build():
    import concourse.mybir as mybir
    import concourse.tile as tile
    from concourse import bacc
    from contextlib import ExitStack

    f32 = mybir.dt.float32
    bf = mybir.dt.bfloat16
    nc = bacc.Bacc(
        "TRN2",
        target_bir_lowering=False,
        debug=False,
        enable_asserts=False,
        num_devices=NCORES,
    )
    aps = {
        "bx0": nc.dram_tensor("bx0", (Dx, 1024), bf, kind="ExternalInput").ap(),
        "bx1": nc.dram_tensor("bx1", (Dx, 512), bf, kind="ExternalInput").ap(),
        "bw": nc.dram_tensor("bw", (P, BW_C), bf, kind="ExternalInput").ap(),
        "bb0": nc.dram_tensor("bb0", (P, 4), f32, kind="ExternalInput").ap(),
        "bb1": nc.dram_tensor("bb1", (P, BB1_C), f32, kind="ExternalInput").ap(),
        "out": nc.dram_tensor("out", (P, NT, 32), f32, kind="ExternalOutput").ap(),
    }
    with tile.TileContext(nc) as tc, ExitStack() as ctx:
        _emit(ctx, tc, nc, aps)
    nc.compile()
    return nc


_compiled_nc = None


def _get_nc():
    global _compiled_nc
    if _compiled_nc is None:
        _compiled_nc = _build()
    return _compiled_nc


def _assemble_host(res):
    """Place device-computed band values into dense zero backgrounds.
    Device image arr[p, t, col]: token a = t*128 + p."""
    S = N + 1
    arr = np.stack([res[i]["out"] for i in range(NCORES)])  # (B,128,8,32)

    def tok(sl):  # (B, 128, 8, 8) -> (B, Dz, N) token-major
        return sl.transpose(0, 3, 2, 1).reshape(B, Dz, N)

    d = tok(arr[:, :, :, 0:8])
    u = tok(arr[:, :, :, 8:16])
    pdiag = tok(arr[:, :, :, 16:24])
    poff = tok(arr[:, :, :, 24:32])[:, :, :N - 1]
    D = np.zeros((B, Dz, N, N), np.float32)
    D.reshape(B, Dz, N * N)[:, :, ::S] = d
    Bm = np.zeros((B, Dz, N, N), np.float32)
    Bm.reshape(B, Dz, N * N)[:, :, ::S] = d
    Bm.reshape(B, Dz, N * N)[:, :, 1::S] = u[:, :, :N - 1]
    Pr = np.zeros((B, Dz, N, N), np.float32)
    Pr.reshape(B, Dz, N * N)[:, :, ::S] = pdiag
    Pr.reshape(B, Dz, N * N)[:, :, 1::S] = poff
    Pr.reshape(B, Dz, N * N)[:, :, N::S] = poff
    return D, Bm, Pr


def _run(trace=False, **inputs):
    from concourse.bass_utils import run_bass_kernel_spmd

    nc = _get_nc()
    x = np.asarray(inputs["x"], dtype=np.float32)
    w = {k: np.asarray(inputs[k], dtype=np.float32) for k in _WEIGHT_SHAPES}
    bw, bb0, bb1 = _pack_weights(w)
    in_maps = []
    for i in range(NCORES):
        bx0, bx1 = _pack_x(x[i], w)
        in_maps.append({"bx0": bx0, "bx1": bx1, "bw": bw,
                        "bb0": bb0, "bb1": bb1})
    out = run_bass_kernel_spmd(
        nc, in_maps, core_ids=list(range(NCORES)), trace=trace)
    return _assemble_host(out.results), out


def kernel(**inputs):
    outs, _ = _run(trace=False, **inputs)
    return outs


def kernel_profiled(**inputs):
    """Like kernel() but with NTFF tracing; returns (outputs, results).
    Falls back to untraced execution when the axon NTFF hook is missing."""
    try:
        return _run(trace=True, **inputs)
    except ModuleNotFoundError:
        return _run(trace=False, **inputs)


# revision 18
# speedup vs baseline: 1.0028x; 1.0028x over previous
"""Trainium2 Bass kernel for nn_EncoderPrecision.

Math: two tiny MLPs map x (B,N,Dx) -> (B,N,Dz); vectors d=exp(mlp_d),
u=mlp_o give structurally sparse outputs with closed-form bands:
  D[i,i] = d_i;  Bmat = D + superdiag(u[:-1]);
  precision[i,i] = d_i^2 + u_{i-1}^2 + eps;  precision[i,i+1] = d_i u_i.

Sharding: data-parallel over batch B=8, one batch element per core;
weights replicated. The device computes every band VALUE (d, u, poff,
pdiag) and ships them as one compact (128, NT, 32) image per core; the
host places those values into dense zero backgrounds (pure layout).

Layout: the HOST pre-transposes x to xT (Dx, N) bf16 so tokens live on
the free dim end-to-end (no device transposes). All matmul operands are
bf16 (end-to-end rel_l2 ~4e-3 vs the 2e-2 gate); PSUM accumulation and
band math stay f32. Inputs arrive via parallel DMA paths (SP-HWDGE:
x+W0 split in two, ACT-HWDGE: W1+W2, Pool-SWDGE: biases+eps). b2 and
the EPS constant are preloaded into PSUM by tiny PE matmuls (ones-row
stationary) so the L3/shift matmuls accumulate onto them and the
ACT/DVE drain engines (the throughput bottleneck alongside PE) carry no
extra copies. The superdiagonal shift u2[a-1] (token a = t*128+p,
tile-major) is a partition shift done with two tiny PE matmuls
(sub-diagonal mask + row127->col0 seam mask). PSUM banks are managed
explicitly (8 x [128,512] tiles). Output ships per 512-token chunk:
chunk 0 via Pool-SWDGE (keeps HWDGE free), chunk 1 via SP.
"""

import numpy as np

EPS = 0.001
B, N, Dx, H, Dz = 8, 1024, 32, 256, 8
NCORES = 8
P = 128
NT = N // P          # 8 token tiles of 128
CH = 512             # chunk (psum bank limit)
NC = N // CH         # 2 chunks
TC = CH // P         # 4 tiles per chunk
KH = H // P          # 2 contraction tiles for H=256

_WEIGHT_SHAPES = {
    "dW0": (Dx, H), "db0": (H,), "dW1": (H, H), "db1": (H,),
    "dW2": (H, Dz), "db2": (Dz,),
    "oW0": (Dx, H), "ob0": (H,), "oW1": (H, H), "ob1": (H,),
    "oW2": (H, Dz), "ob2": (Dz,),
}

# bx0 bf16 (32, 1024): xT[:, 0:512] | w0d(256) | w0o(256)
# bx1 bf16 (32, 512):  xT[:, 512:1024]
# bw  bf16 (128, 1056): w1d(512) | w1o(512) | w2d(16) | w2o(16)
# bb0 f32  (128, 4):   b0d(2) b0o(2)            [first: gates L1 relus]
# bb1 f32  (128, 100): b1d(2) b1o(2) b2bc(64) epsbc(32)
BW_C = 512 + 512 + 16 + 16
BB1_C = 2 + 2 + 64 + 32


def _bf16():
    import ml_dtypes
    return np.dtype(ml_dtypes.bfloat16)


def _pack_weights(w):
    bf16 = _bf16()
    bw = np.zeros((P, BW_C), bf16)
    bw[:, 0:512] = w["dW1"].reshape(KH, P, H).transpose(1, 0, 2).reshape(P, 512).astype(bf16)
    bw[:, 512:1024] = w["oW1"].reshape(KH, P, H).transpose(1, 0, 2).reshape(P, 512).astype(bf16)
    bw[:, 1024:1040] = w["dW2"].reshape(KH, P, Dz).transpose(1, 0, 2).reshape(P, 16).astype(bf16)
    bw[:, 1040:1056] = w["oW2"].reshape(KH, P, Dz).transpose(1, 0, 2).reshape(P, 16).astype(bf16)
    bb0 = np.zeros((P, 4), np.float32)
    bb0[:, 0:2] = w["db0"].reshape(KH, P).T
    bb0[:, 2:4] = w["ob0"].reshape(KH, P).T
    bb1 = np.zeros((P, BB1_C), np.float32)
    bb1[:, 0:2] = w["db1"].reshape(KH, P).T
    bb1[:, 2:4] = w["ob1"].reshape(KH, P).T
    b2bc = np.concatenate([w["db2"], w["ob2"]])  # (16,)
    bb1[:, 4:68] = np.tile(b2bc, 4)[None, :]
    bb1[:, 68:100] = EPS
    return bw, bb0, bb1


def _pack_x(xi, w):
    bf16 = _bf16()
    xt = np.ascontiguousarray(xi.T).astype(bf16)
    bx0 = np.zeros((Dx, 1024), bf16)
    bx0[:, 0:512] = xt[:, 0:512]
    bx0[:, 512:768] = w["dW0"].astype(bf16)
    bx0[:, 768:1024] = w["oW0"].astype(bf16)
    bx1 = np.ascontiguousarray(xt[:, 512:1024])
    return bx0, bx1


def _emit(ctx, tc, nc, aps):
    import concourse.mybir as mybir

    f32 = mybir.dt.float32
    bf = mybir.dt.bfloat16
    AF = mybir.ActivationFunctionType

    const = ctx.enter_context(tc.tile_pool(name="const", bufs=1))
    work = ctx.enter_context(tc.tile_pool(name="work", bufs=1))
    bands = ctx.enter_context(tc.tile_pool(name="bands", bufs=1))
    psum = ctx.enter_context(tc.tile_pool(name="psum", bufs=1, space="PSUM"))

    pb = [psum.tile([P, CH], f32, tag=f"pb{i}", name=f"pb{i}")
          for i in range(8)]

    # warmup: set pe_busy_start early so the 3us p-state ramp completes
    # before the real matmul stream begins
    zp = const.tile([1, 8], f32)
    nc.vector.memset(zp[:], 0.0)
    wps = pb[7]
    nc.tensor.matmul(wps[:1, 0:8], zp[:, 0:1], zp[:], start=True, stop=True)

    # --- input DMAs on parallel issue paths, x+w0 first ---
    bx_sb = const.tile([Dx, 1024], bf)   # xT[:, 0:512] | w0d | w0o
    x1_sb = const.tile([Dx, 512], bf)    # xT[:, 512:1024]
    nc.sync.dma_start(bx_sb[:], aps["bx0"][:])
    nc.scalar.dma_start(x1_sb[:], aps["bx1"][:])                 # ACT HWDGE
    bw_sb = const.tile([P, BW_C], bf)
    nc.sync.dma_start(bw_sb[:], aps["bw"][:])                    # SP second
    bb0_sb = const.tile([P, 4], f32)
    nc.gpsimd.dma_start(bb0_sb[:], aps["bb0"][:])                # Pool SWDGE
    bb_sb = const.tile([P, BB1_C], f32)
    nc.gpsimd.dma_start(bb_sb[:], aps["bb1"][:])

    xv = [bx_sb[:, 0:512], x1_sb[:]]
    w0 = {"d": bx_sb[:, 512:768], "o": bx_sb[:, 768:1024]}
    w1 = {"d": bw_sb[:, 0:512].rearrange("p (ko m) -> p ko m", ko=KH),
          "o": bw_sb[:, 512:1024].rearrange("p (ko m) -> p ko m", ko=KH)}
    w2 = {"d": bw_sb[:, 1024:1040].rearrange("p (ko m) -> p ko m", ko=KH),
          "o": bw_sb[:, 1040:1056].rearrange("p (ko m) -> p ko m", ko=KH)}
    b0 = {"d": bb0_sb[:, 0:2], "o": bb0_sb[:, 2:4]}
    b1 = {"d": bb_sb[:, 0:2], "o": bb_sb[:, 2:4]}

    # masks (Pool, after its DMA issue; needed only at band time)
    # SH[k, m] = 1 iff k = m-1  ->  (SH^T v)[m] = v[m-1], 0 at m=0
    sh = const.tile([P, P], f32)
    nc.gpsimd.memset(sh[:], 0.0)
    nc.gpsimd.affine_select(
        out=sh[:], in_=sh[:], compare_op=mybir.AluOpType.not_equal,
        fill=1.0, base=1, pattern=[[-1, P]], channel_multiplier=1)
    # S127[k, m] = 1 iff k = 127 and m = 0  (tile-seam: prev tile's p=127)
    s127 = const.tile([P, P], f32)
    nc.gpsimd.memset(s127[:], 0.0)
    nc.gpsimd.affine_select(
        out=s127[:], in_=s127[:], compare_op=mybir.AluOpType.not_equal,
        fill=1.0, base=127, pattern=[[1, P]], channel_multiplier=-1)
    ones = const.tile([1, P], f32)
    nc.gpsimd.memset(ones[:], 1.0)

    h0, h1 = {}, {}
    for br in ("d", "o"):
        h0[br] = work.tile([P, KH, N], bf, tag=f"h0{br}", name=f"h0{br}")
        h1[br] = work.tile([P, KH, N], bf, tag=f"h1{br}", name=f"h1{br}")

    obig = bands.tile([P, NT, 32], f32)   # d | u | pdiag | poff per token
    d2 = bands.tile([P, NT, Dz], f32)
    u2 = bands.tile([P, NT, Dz], f32)

    def relu_store(eng, dst, ps, bias):
        if eng == "A":
            nc.scalar.activation(dst, ps, AF.Relu, bias=bias)
        else:
            nc.vector.tensor_scalar(
                dst, ps, bias, 0.0, mybir.AluOpType.add, mybir.AluOpType.max)

    def emit_l1(c):
        ns = slice(c * CH, (c + 1) * CH)
        for bi, br in enumerate(("d", "o")):
            for m in range(KH):
                ps = pb[4 * c + 2 * bi + m]
                nc.tensor.matmul(
                    ps[:], w0[br][:, m * P:(m + 1) * P], xv[c][:],
                    start=True, stop=True)
                relu_store("A" if m == 0 else "D",
                           h0[br][:, m, ns], ps[:], b0[br][:, m:m + 1])

    def emit_l2(c):
        ns = slice(c * CH, (c + 1) * CH)
        for bi, br in enumerate(("d", "o")):
            for m in range(KH):
                ps = pb[4 * c + 2 * bi + m]
                for k in range(KH):
                    nc.tensor.matmul(
                        ps[:], w1[br][:, k, m * P:(m + 1) * P],
                        h0[br][:, k, ns],
                        start=(k == 0), stop=(k == KH - 1))
                relu_store("A" if m == 0 else "D",
                           h1[br][:, m, ns], ps[:], b1[br][:, m:m + 1])

    def emit_l3(c):
        # b2 preload by PE (ones-row matmul), then head matmuls accumulate
        yv = pb[2 * c][:, 0:TC * 16].rearrange("p (t z) -> p t z", z=16)
        nc.tensor.matmul(yv[:], ones[:], bb_sb[0:1, 4:68],
                         start=True, stop=False)
        for tt in range(TC):
            ta = c * TC + tt
            for bi, br in enumerate(("d", "o")):
                zs = slice(bi * Dz, (bi + 1) * Dz)
                for k in range(KH):
                    nc.tensor.matmul(
                        yv[:, tt, zs], h1[br][:, k, ta * P:(ta + 1) * P],
                        w2[br][:, k, :],
                        start=False, stop=(k == KH - 1))
        return yv

    def emit_shift(c):
        # sps = EPS + u2 shifted by one token (partition shift + tile seam)
        sv = pb[2 * c + 1][:, 0:TC * Dz].rearrange("p (t z) -> p t z", z=Dz)
        nc.tensor.matmul(sv[:], ones[:], bb_sb[0:1, 68:100],
                         start=True, stop=False)
        ts = slice(c * TC, (c + 1) * TC)
        nc.tensor.matmul(sv[:, 0:TC, :], sh[:], u2[:, ts, :],
                         start=False, stop=False)
        if c == 0:
            nc.tensor.matmul(sv[:, 1:TC, :], s127[:], u2[:, 0:TC - 1, :],
                             start=False, stop=True)
        else:
            nc.tensor.matmul(sv[:, 0:TC, :], s127[:],
                             u2[:, c * TC - 1:(c + 1) * TC - 1, :],
                             start=False, stop=True)
        return sv

    import bass_rust as _br

    prev = {}

    def pin(key, bi):
        # nosync edge: same-engine queue-order pin across band ops
        if key in prev:
            _br.add_dep_helper(bi.ins, prev[key].ins, sync=False,
                               reason="band order pin")
        prev[key] = bi

    def emit_bands1(c, yv):
        # ACT: d, d2 (both Exp); DVE: u copy-out; Pool: u2, poff.
        ts = slice(c * TC, (c + 1) * TC)
        pin("A", nc.scalar.activation(obig[:, ts, 0:Dz], yv[:, :, 0:Dz], AF.Exp))
        pin("V", nc.vector.tensor_copy(obig[:, ts, Dz:16], yv[:, :, Dz:16]))
        pin("P", nc.gpsimd.tensor_mul(
            u2[:, ts, :], obig[:, ts, Dz:16], obig[:, ts, Dz:16]))
        pin("A", nc.scalar.activation(d2[:, ts, :], yv[:, :, 0:Dz], AF.Exp,
                                      scale=2.0))
        pin("P", nc.gpsimd.tensor_mul(
            obig[:, ts, 24:32], obig[:, ts, 0:Dz], obig[:, ts, Dz:16]))

    def emit_bands2(c, sv):
        ts = slice(c * TC, (c + 1) * TC)
        pin("V", nc.vector.tensor_tensor(
            obig[:, ts, 16:24], d2[:, ts, :], sv[:, 0:TC, :],
            mybir.AluOpType.add))

    emit_l1(0)
    emit_l1(1)
    emit_l2(0)
    emit_l2(1)
    yv0 = emit_l3(0)
    yv1 = emit_l3(1)
    emit_bands1(0, yv0)
    emit_bands1(1, yv1)
    sv0 = emit_shift(0)
    sv1 = emit_shift(1)
    emit_bands2(0, sv0)
    emit_bands2(1, sv1)
    nc.sync.dma_start(aps["out"][:], obig[:])


def _build():
    import concourse.mybir as mybir
    import concourse.tile as tile
    from concourse import bacc
    from contextlib import ExitStack

    f32 = mybir.dt.float32
    bf = mybir.dt.bfloat16
    nc = bacc.Bacc(
        "TRN2",
        target_bir_lowering=False,
        debug=False,
        enable_asserts=False,
        num_devices=NCORES,
    )
    aps = {
        "bx0": nc.dram_tensor("bx0", (Dx, 1024), bf, kind="ExternalInput").ap(),
        "bx1": nc.dram_tensor("bx1", (Dx, 512), bf, kind="ExternalInput").ap(),
        "bw": nc.dram_tensor("bw", (P, BW_C), bf, kind="ExternalInput").ap(),
        "bb0": nc.dram_tensor("bb0", (P, 4), f32, kind="ExternalInput").ap(),
        "bb1": nc.dram_tensor("bb1", (P, BB1_C), f32, kind="ExternalInput").ap(),
        "out": nc.dram_tensor("out", (P, NT, 32), f32, kind="ExternalOutput").ap(),
    }
    with tile.TileContext(nc) as tc, ExitStack() as ctx:
        _emit(ctx, tc, nc, aps)
    nc.compile()
    return nc


_compiled_nc = None


def _get_nc():
    global _compiled_nc
    if _compiled_nc is None:
        _compiled_nc = _build()
    return _compiled_nc


def _assemble_host(res):
    """Place device-computed band values into dense zero backgrounds.
    Device image arr[p, t, col]: token a = t*128 + p."""
    S = N + 1
    arr = np.stack([res[i]["out"] for i in range(NCORES)])  # (B,128,8,32)

    def tok(sl):  # (B, 128, 8, 8) -> (B, Dz, N) token-major
        return sl.transpose(0, 3, 2, 1).reshape(B, Dz, N)

    d = tok(arr[:, :, :, 0:8])
    u = tok(arr[:, :, :, 8:16])
    pdiag = tok(arr[:, :, :, 16:24])
    poff = tok(arr[:, :, :, 24:32])[:, :, :N - 1]
    D = np.zeros((B, Dz, N, N), np.float32)
    D.reshape(B, Dz, N * N)[:, :, ::S] = d
    Bm = np.zeros((B, Dz, N, N), np.float32)
    Bm.reshape(B, Dz, N * N)[:, :, ::S] = d
    Bm.reshape(B, Dz, N * N)[:, :, 1::S] = u[:, :, :N - 1]
    Pr = np.zeros((B, Dz, N, N), np.float32)
    Pr.reshape(B, Dz, N * N)[:, :, ::S] = pdiag
    Pr.reshape(B, Dz, N * N)[:, :, 1::S] = poff
    Pr.reshape(B, Dz, N * N)[:, :, N::S] = poff
    return D, Bm, Pr


def _run(trace=False, **inputs):
    from concourse.bass_utils import run_bass_kernel_spmd

    nc = _get_nc()
    x = np.asarray(inputs["x"], dtype=np.float32)
    w = {k: np.asarray(inputs[k], dtype=np.float32) for k in _WEIGHT_SHAPES}
    bw, bb0, bb1 = _pack_weights(w)
    in_maps = []
    for i in range(NCORES):
        bx0, bx1 = _pack_x(x[i], w)
        in_maps.append({"bx0": bx0, "bx1": bx1, "bw": bw,
                        "bb0": bb0, "bb1": bb1})
    out = run_bass_kernel_spmd(
        nc, in_maps, core_ids=list(range(NCORES)), trace=trace)
    return _assemble_host(out.results), out


def kernel(**inputs):
    outs, _ = _run(trace=False, **inputs)
    return outs


def kernel_profiled(**inputs):
    """Like kernel() but with NTFF tracing; returns (outputs, results).
    Falls back to untraced execution when the axon NTFF hook is missing."""
    try:
        return _run(trace=True, **inputs)
    except ModuleNotFoundError:
        return _run(trace=False, **inputs)
